# revision 28
# baseline (speedup 1.0000x reference)
import os
import sys

import numpy as np

for _p in ("/opt/trn_rl_repo",):
    if _p not in sys.path:
        sys.path.insert(0, _p)

LAST_EXEC_NS = None
_DEBUG_NO_COLLECTIVE = os.environ.get("BASS_DEBUG_NO_COLLECTIVE") == "1"
_DEBUG_CUT = int(os.environ.get("BASS_DEBUG_CUT", "0"))

# Problem constants (hardcoded; kernel.py must be self-contained)
B = 512
N_CORES = 8
BC = B // N_CORES          # 64 images per core
NCHUNK = 4
BCH = BC // NCHUNK         # 16 images per chunk
C = 3
H = W = 128
OY = OX = 26
M = 3 * OY                 # 78 (m = 3*oy + co)
EPS = 1e-5
INV_N = 1.0 / (B * H * W)  # global BN count per channel

_PROG = None


def _build_program():
    import concourse.bass as bass
    import concourse.mybir as mybir
    import concourse.tile as tile
    import bass_rust as _bass_rust

    f32 = mybir.dt.float32
    AF = mybir.ActivationFunctionType
    ALU = mybir.AluOpType
    AX = mybir.AxisListType

    nc = bass.Bass(trn_type="TRN2", num_devices=N_CORES)

    xc_d = nc.dram_tensor("xc", [BC, C, H, W], f32, kind="ExternalInput")
    convl_d = nc.dram_tensor("convl", [128, 15, M], f32, kind="ExternalInput")
    w2bd_d = nc.dram_tensor("w2bd", [M, M], f32, kind="ExternalInput")
    g_d = nc.dram_tensor("gmat", [M, OX, 128], f32, kind="ExternalInput")
    aux4_d = nc.dram_tensor("aux4", [4, 6, M], f32, kind="ExternalInput")
    b2m_d = nc.dram_tensor("b2m", [M, 1], f32, kind="ExternalInput")
    fc1b_d = nc.dram_tensor("fc1b", [128, 1], f32, kind="ExternalInput")
    fc2wt_d = nc.dram_tensor("fc2wt", [128, 10], f32, kind="ExternalInput")
    fc2b_d = nc.dram_tensor("fc2b", [1, 10], f32, kind="ExternalInput")
    gb3_d = nc.dram_tensor("gb3", [C, 2], f32, kind="ExternalInput")
    out_d = nc.dram_tensor("out", [BC, 10], f32, kind="ExternalOutput")

    with tile.TileContext(nc) as tc:
        with (
            tc.tile_pool(name="wp", bufs=1) as wp,
            tc.tile_pool(name="xp", bufs=4) as xp,
            tc.tile_pool(name="scrp", bufs=1) as scrp,
            tc.tile_pool(name="up", bufs=1) as up,
            tc.tile_pool(name="stp", bufs=2) as stp,
            tc.tile_pool(name="smallp", bufs=1) as smallp,
            tc.tile_pool(name="dp", bufs=1, space="DRAM") as dp,
        ):
            # ---- weights/constants to SBUF ----
            convl_sb = wp.tile([128, 15, M], f32)
            nc.sync.dma_start(convl_sb[:], convl_d[:])
            w2bd_sb = wp.tile([M, M], f32)
            nc.sync.dma_start(w2bd_sb[:], w2bd_d[:])
            g_sb = wp.tile([M, OX, 128], f32)
            nc.sync.dma_start(g_sb[:], g_d[:])
            aux4_sb = wp.tile([4, 6, M], f32)
            nc.sync.dma_start(aux4_sb[:], aux4_d[:])
            b2m_sb = wp.tile([M, 1], f32)
            nc.sync.dma_start(b2m_sb[:], b2m_d[:])
            fc1b_sb = wp.tile([128, 1], f32)
            nc.sync.dma_start(fc1b_sb[:], fc1b_d[:])
            fc2wt_sb = wp.tile([128, 10], f32)
            nc.sync.dma_start(fc2wt_sb[:], fc2wt_d[:])
            fc2b_sb = wp.tile([1, 10], f32)
            nc.sync.dma_start(fc2b_sb[:], fc2b_d[:])
            gb3_sb = wp.tile([C, 2], f32)
            nc.sync.dma_start(gb3_sb[:], gb3_d[:])
            ones64_sb = wp.tile([1, BC], f32)
            nc.vector.memset(ones64_sb[:], 1.0)
            ones128_sb = wp.tile([128, 1], f32)
            nc.vector.memset(ones128_sb[:], 1.0)
            acc6 = wp.tile([128, 6], f32)
            nc.vector.memset(acc6[:], 0.0)

            # U accumulators: [m, ci, b, ox]
            u_sb = up.tile([M, C, BC, OX], f32)

            # ---- main pass over batch chunks ----
            with tc.tile_pool(name="pscv", bufs=3, space="PSUM") as pscv:
                for c in range(NCHUNK):
                    x_sb = xp.tile([128, BCH, C, 130], f32)
                    nc.vector.memset(x_sb[:, :, :, 0:1], 0.0)
                    nc.vector.memset(x_sb[:, :, :, 129:130], 0.0)
                    nc.sync.dma_start(
                        x_sb[:, :, :, 1:129],
                        xc_d[c * BCH : (c + 1) * BCH].rearrange("b c h w -> h b c w"),
                    )
                    x5 = x_sb.rearrange("h b c (ox f) -> h b c ox f", f=5)
                    cs = stp.tile([128, 6], f32)
                    for ci in range(C):
                        ps = pscv.tile([M, 512], f32)
                        out_ap = ps[:, 0 : BCH * OX].rearrange(
                            "m (b ox) -> m b ox", b=BCH
                        )
                        for kx in range(5):
                            nc.tensor.matmul(
                                out_ap,
                                convl_sb[:, ci * 5 + kx, :],
                                x5[:, :, ci, :, kx],
                                start=(kx == 0),
                                stop=(kx == 4),
                            )
                        nc.vector.tensor_copy(
                            u_sb[:, ci, c * BCH : (c + 1) * BCH, :], out_ap
                        )
                        xin = x_sb[:, :, ci, 1:129]
                        sq_scr = scrp.tile([128, BCH, 128], f32)
                        nc.scalar.activation(
                            sq_scr[:], xin, AF.Square,
                            accum_out=cs[:, 3 + ci : 4 + ci],
                        )
                        sx_scr = scrp.tile([128, BCH, 128], f32)
                        nc.vector.tensor_scalar(
                            sx_scr[:], xin, 0.0, 0.0, ALU.add, ALU.add,
                            accum_out=cs[:, ci : ci + 1],
                        )
                    nc.vector.tensor_add(acc6[:], acc6[:], cs[:])

            # ---- global stats: partition-reduce + AllReduce ----
            with tc.tile_pool(name="psst", bufs=1, space="PSUM") as psst:
                st_ps = psst.tile([6, 1], f32)
                nc.tensor.matmul(st_ps[:], acc6[:], ones128_sb[:], start=True, stop=True)
                st_sb = smallp.tile([8, 1], f32)
                nc.vector.memset(st_sb[:], 0.0)
                nc.vector.tensor_copy(st_sb[0:6, :], st_ps[:])

            g6 = smallp.tile([8, 1], f32)
            if _DEBUG_NO_COLLECTIVE:
                cc_in = dp.tile([8, 1], f32)
                cc_out = dp.tile([8, 1], f32)
                nc.sync.dma_start(cc_in[:], st_sb[:])
                nc.sync.dma_start(cc_out[:], cc_in[:])
                nc.sync.dma_start(g6[:], cc_out[:])
            else:
                # AllGather (bypass) + local matmul-sum: avoids the AllReduce
                # path that raises NRT_EXEC_UNIT_UNRECOVERABLE here.
                cc_in = dp.tile([8, 1], f32)
                cc_out = dp.tile([N_CORES * 8, 1], f32)
                nc.gpsimd.dma_start(cc_in[:], st_sb[:])
                nc.gpsimd.collective_compute(
                    "AllGather",
                    ALU.bypass,
                    replica_groups=[list(range(N_CORES))],
                    ins=[cc_in.opt()],
                    outs=[cc_out.opt()],
                )
                cg8 = smallp.tile([N_CORES, 8], f32)
                nc.sync.dma_start(
                    cg8[:], cc_out[:].rearrange("(r s) one -> r (s one)", r=N_CORES)
                )
                with tc.tile_pool(name="psag", bufs=1, space="PSUM") as psag:
                    g6ps = psag.tile([8, 8], f32)
                    nc.tensor.matmul(
                        g6ps[:, 0:1], cg8[:],
                        ones128_sb[0:8, :], start=True, stop=True,
                    )
                    nc.vector.tensor_copy(g6[:], g6ps[:, 0:1])

            lvl = _DEBUG_CUT if _DEBUG_CUT else 99
            if lvl <= 8:
                dummy = smallp.tile([BC, 10], f32)
                nc.vector.memset(dummy[:], 0.0)
                nc.sync.dma_start(out_d[:], dummy[:])

            # ---- s/t per channel ----
            if lvl > 1:
                ex6 = smallp.tile([6, 1], f32)
                nc.scalar.activation(ex6[:], g6[0:6, :], AF.Copy, bias=0.0, scale=INV_N)
                # compute engines need partition-start % 32 == 0; DMA rows 3:6 down
                esq3 = smallp.tile([C, 1], f32)
                nc.sync.dma_start(esq3[:], ex6[3:6, :])
                msq = smallp.tile([C, 1], f32)
                nc.vector.tensor_mul(msq[:], ex6[0:3, :], ex6[0:3, :])
                var3 = smallp.tile([C, 1], f32)
                nc.vector.tensor_sub(var3[:], esq3[:], msq[:])
                eps_sb = smallp.tile([C, 1], f32)
                nc.vector.memset(eps_sb[:], EPS)
                std3 = smallp.tile([C, 1], f32)
                nc.scalar.activation(std3[:], var3[:], AF.Sqrt, bias=eps_sb[:])
                rinv3 = smallp.tile([C, 1], f32)
                nc.vector.reciprocal(rinv3[:], std3[:])
                s3 = smallp.tile([C, 1], f32)
                nc.vector.tensor_mul(s3[:], rinv3[:], gb3_sb[:, 0:1])
                mt3 = smallp.tile([C, 1], f32)
                nc.vector.tensor_mul(mt3[:], ex6[0:3, :], s3[:])
                t3 = smallp.tile([C, 1], f32)
                nc.vector.tensor_sub(t3[:], gb3_sb[:, 1:2], mt3[:])
                rt4 = smallp.tile([4, 1], f32)
                nc.vector.memset(rt4[:], 1.0)
                nc.vector.tensor_copy(rt4[0:3, :], t3[:])
                rs4 = smallp.tile([4, 1], f32)
                nc.vector.memset(rs4[:], 1.0)
                nc.vector.tensor_copy(rs4[0:3, :], s3[:])

            # ---- broadcast vectors via K=4 matmuls ----
            # cols: 0=Tm_main 1=Tm_ox0 2=Tm_ox25 3..5 = s_bcast per ci
            if lvl > 2:
                vec_sb = smallp.tile([M, 6], f32)
                with tc.tile_pool(name="pstn", bufs=1, space="PSUM") as pstn:
                    tn_ps = pstn.tile([M, 8], f32)
                    for j in range(6):
                        rhs = rt4 if j < 3 else rs4
                        nc.tensor.matmul(
                            tn_ps[:, j : j + 1], aux4_sb[:, j, :], rhs[:],
                            start=True, stop=True,
                        )
                    nc.vector.tensor_copy(vec_sb[:], tn_ps[:, 0:6])

            # ---- assemble z1 = relu(y1) ----
            if lvl > 3:
                z1 = up.tile([M, BC, OX], f32)
                nc.vector.tensor_scalar(
                    z1[:], u_sb[:, 0], vec_sb[:, 3:4], vec_sb[:, 0:1], ALU.mult, ALU.add
                )
                nc.vector.scalar_tensor_tensor(
                    z1[:], u_sb[:, 1], vec_sb[:, 4:5], z1[:], ALU.mult, ALU.add
                )
                nc.vector.scalar_tensor_tensor(
                    z1[:], u_sb[:, 2], vec_sb[:, 5:6], z1[:], ALU.mult, ALU.add
                )
                nc.vector.tensor_scalar(
                    z1[:, :, 0:1], z1[:, :, 0:1], vec_sb[:, 1:2], None, ALU.add
                )
                nc.vector.tensor_scalar(
                    z1[:, :, 25:26], z1[:, :, 25:26], vec_sb[:, 2:3], None, ALU.add
                )
                nc.vector.tensor_scalar_max(z1[:], z1[:], 0.0)

            # ---- z2 = relu(W2 . z1 + b2) (1x1 conv over co) ----
            if lvl > 4:
                z2 = up.tile([M, BC, OX], f32)
                with tc.tile_pool(name="psw2", bufs=2, space="PSUM") as psw2:
                    for j in range(4):
                        pw = psw2.tile([M, 512], f32)
                        w_ap = pw[:, 0 : BCH * OX].rearrange("m (b ox) -> m b ox", b=BCH)
                        nc.tensor.matmul(
                            w_ap, w2bd_sb[:], z1[:, j * BCH : (j + 1) * BCH, :],
                            start=True, stop=True,
                        )
                        nc.scalar.activation(
                            z2[:, j * BCH : (j + 1) * BCH, :], w_ap, AF.Relu,
                            bias=b2m_sb[:],
                        )

            # ---- fc1 (gathered sparse cols) + relu ----
            if lvl > 5:
                f_sb = smallp.tile([128, BC], f32)
                with tc.tile_pool(name="psf", bufs=1, space="PSUM") as psf:
                    f_ps = psf.tile([128, BC], f32)
                    for ox in range(OX):
                        nc.tensor.matmul(
                            f_ps[:], g_sb[:, ox, :], z2[:, :, ox],
                            start=(ox == 0), stop=(ox == OX - 1),
                        )
                    nc.scalar.activation(f_sb[:], f_ps[:], AF.Relu, bias=fc1b_sb[:])

            # ---- fc2 + log_softmax ----
            if lvl > 6:
                with tc.tile_pool(name="pso", bufs=1, space="PSUM") as pso:
                    o_ps = pso.tile([BC, 16], f32)
                    lg = o_ps[:, 0:10]
                    nc.tensor.matmul(lg, f_sb[:], fc2wt_sb[:], start=True, stop=False)
                    nc.tensor.matmul(lg, ones64_sb[:], fc2b_sb[:], start=False, stop=True)
                    if lvl > 7:
                        mx = smallp.tile([BC, 1], f32)
                        nc.vector.tensor_reduce(mx[:], lg, AX.X, ALU.max)
                        nmx = smallp.tile([BC, 1], f32)
                        nc.scalar.activation(nmx[:], mx[:], AF.Copy, bias=0.0, scale=-1.0)
                        exps = smallp.tile([BC, 10], f32)
                        se = smallp.tile([BC, 1], f32)
                        nc.scalar.activation(exps[:], lg, AF.Exp, bias=nmx[:], accum_out=se[:])
                        ls = smallp.tile([BC, 1], f32)
                        nc.scalar.activation(ls[:], se[:], AF.Ln)
                    if lvl > 8:
                        out_sb = smallp.tile([BC, 10], f32)
                        nc.vector.tensor_scalar(
                            out_sb[:], lg, mx[:], ls[:], ALU.subtract, ALU.subtract
                        )
                        nc.sync.dma_start(out_d[:], out_sb[:])

    if not nc.is_finalized():
        nc.finalize()
    # Walrus allows at most 1 sem wait per instruction (2 on event sems).
    # Run the bacc split passes the BIR path doesn't apply by default.
    _bass_rust.move_matmul_waits_to_ldweights(nc.m)
    _bass_rust.generate_event_semaphores(nc)
    return nc


def _host_aux(w1, b1, w2, b2, fc1_w, fc1_b, fc2_w, fc2_b, gamma, beta):
    f = np.float32
    convl = np.zeros((128, 15, M), f)
    for ci in range(C):
        for kx in range(5):
            idx = ci * 5 + kx
            for oy in range(OY):
                for ky in range(5):
                    h = 5 * oy + ky - 1
                    if 0 <= h < 128:
                        convl[h, idx, 3 * oy : 3 * oy + 3] = w1[:, ci, ky, kx]

    w2bd = np.zeros((M, M), f)
    blk = w2[:, :, 0, 0].T.astype(f)  # [co', co]
    for oy in range(OY):
        w2bd[3 * oy : 3 * oy + 3, 3 * oy : 3 * oy + 3] = blk

    oyv = np.arange(OY)
    py = (5 * oyv) // 2
    px = py
    cols = (
        np.arange(C)[:, None, None] * 65 * 65
        + py[None, :, None] * 65
        + px[None, None, :]
    )  # [co, oy, ox]
    gmat = (
        fc1_w[:, cols].transpose(2, 1, 3, 0).reshape(M, OX, 128).astype(f)
    )  # [m=(oy,co), ox, k]

    full = w1.sum(axis=(2, 3))       # [co, ci]
    row0 = w1[:, :, 0, :].sum(-1)
    row4 = w1[:, :, 4, :].sum(-1)
    col0 = w1[:, :, :, 0].sum(-1)
    col4 = w1[:, :, :, 4].sum(-1)

    aux4 = np.zeros((4, 6, M), f)
    for m in range(M):
        oy, co = divmod(m, 3)
        for ci in range(C):
            aux4[ci, 0, m] = (
                full[co, ci]
                - (oy == 0) * row0[co, ci]
                - (oy == OY - 1) * row4[co, ci]
            )
            aux4[ci, 1, m] = (
                -col0[co, ci]
                + (oy == 0) * w1[co, ci, 0, 0]
                + (oy == OY - 1) * w1[co, ci, 4, 0]
            )
            aux4[ci, 2, m] = (
                -col4[co, ci]
                + (oy == 0) * w1[co, ci, 0, 4]
                + (oy == OY - 1) * w1[co, ci, 4, 4]
            )
            aux4[ci, 3 + ci, m] = 1.0
        aux4[3, 0, m] = b1[co]

    b2m = np.repeat(b2[None, :], OY, axis=0).reshape(M, 1).astype(f)
    fc1b = fc1_b.reshape(128, 1).astype(f)
    fc2wt = np.ascontiguousarray(fc2_w.T).astype(f)
    fc2b_a = fc2_b.reshape(1, 10).astype(f)
    gb3 = np.stack([gamma, beta], axis=1).astype(f)
    return dict(
        convl=np.ascontiguousarray(convl),
        w2bd=np.ascontiguousarray(w2bd),
        gmat=np.ascontiguousarray(gmat),
        aux4=np.ascontiguousarray(aux4),
        b2m=b2m,
        fc1b=fc1b,
        fc2wt=fc2wt,
        fc2b=fc2b_a,
        gb3=np.ascontiguousarray(gb3),
    )


def kernel(**inputs):
    global _PROG, LAST_EXEC_NS
    from concourse.bass_utils import run_bass_kernel_spmd

    x = np.asarray(inputs["x"], dtype=np.float32)
    aux = _host_aux(
        np.asarray(inputs["w1"], np.float32),
        np.asarray(inputs["b1"], np.float32),
        np.asarray(inputs["w2"], np.float32),
        np.asarray(inputs["b2"], np.float32),
        np.asarray(inputs["fc1_w"], np.float32),
        np.asarray(inputs["fc1_b"], np.float32),
        np.asarray(inputs["fc2_w"], np.float32),
        np.asarray(inputs["fc2_b"], np.float32),
        np.asarray(inputs["bn_gamma"], np.float32),
        np.asarray(inputs["bn_beta"], np.float32),
    )

    if _PROG is None:
        _PROG = _build_program()
    nc = _PROG

    in_maps = []
    for cid in range(N_CORES):
        m = {"xc": np.ascontiguousarray(x[cid * BC : (cid + 1) * BC])}
        m.update(aux)
        in_maps.append(m)

    trace = os.environ.get("BASS_KERNEL_TRACE") == "1"
    res = run_bass_kernel_spmd(nc, in_maps, list(range(N_CORES)), trace=trace)
    LAST_EXEC_NS = res.exec_time_ns
    out = np.concatenate([res.results[cid]["out"] for cid in range(N_CORES)], axis=0)
    return out.astype(np.float32)


# revision 31
# speedup vs baseline: 1.2185x; 1.2185x over previous
import os
import sys

import numpy as np

for _p in ("/opt/trn_rl_repo",):
    if _p not in sys.path:
        sys.path.insert(0, _p)

LAST_EXEC_NS = None
_DEBUG_NO_COLLECTIVE = os.environ.get("BASS_DEBUG_NO_COLLECTIVE") == "1"
_DEBUG_CUT = int(os.environ.get("BASS_DEBUG_CUT", "0"))

# Problem constants (hardcoded; kernel.py must be self-contained)
B = 512
N_CORES = 8
BC = B // N_CORES          # 64 images per core
NCHUNK = 4
BCH = BC // NCHUNK         # 16 images per chunk
C = 3
H = W = 128
OY = OX = 26
M = 3 * OY                 # 78 (m = 3*oy + co)
EPS = 1e-5
INV_N = 1.0 / (B * H * W)  # global BN count per channel

_PROG = None


def _build_program():
    import concourse.bass as bass
    import concourse.mybir as mybir
    import concourse.tile as tile
    import bass_rust as _bass_rust

    f32 = mybir.dt.float32
    bf16 = mybir.dt.bfloat16
    AF = mybir.ActivationFunctionType
    ALU = mybir.AluOpType
    AX = mybir.AxisListType

    nc = bass.Bass(trn_type="TRN2", num_devices=N_CORES)

    xc_d = nc.dram_tensor("xc", [BC, C, H, W], f32, kind="ExternalInput")
    convl_d = nc.dram_tensor("convl", [128, 15, M], f32, kind="ExternalInput")
    w2bd_d = nc.dram_tensor("w2bd", [M, M], f32, kind="ExternalInput")
    g_d = nc.dram_tensor("gmat", [M, OX, 128], f32, kind="ExternalInput")
    aux4_d = nc.dram_tensor("aux4", [4, 6, M], f32, kind="ExternalInput")
    b2m_d = nc.dram_tensor("b2m", [M, 1], f32, kind="ExternalInput")
    fc1b_d = nc.dram_tensor("fc1b", [128, 1], f32, kind="ExternalInput")
    fc2wt_d = nc.dram_tensor("fc2wt", [128, 10], f32, kind="ExternalInput")
    fc2b_d = nc.dram_tensor("fc2b", [1, 10], f32, kind="ExternalInput")
    gb3_d = nc.dram_tensor("gb3", [C, 2], f32, kind="ExternalInput")
    out_d = nc.dram_tensor("out", [BC, 10], f32, kind="ExternalOutput")

    with tile.TileContext(nc) as tc:
        with (
            tc.tile_pool(name="wp", bufs=1) as wp,
            tc.tile_pool(name="xp", bufs=4) as xp,
            tc.tile_pool(name="scrp", bufs=1) as scrp,
            tc.tile_pool(name="up", bufs=1) as up,
            tc.tile_pool(name="stp", bufs=2) as stp,
            tc.tile_pool(name="smallp", bufs=1) as smallp,
            tc.tile_pool(name="dp", bufs=1, space="DRAM") as dp,
        ):
            # ---- weights/constants to SBUF ----
            convl_sb = wp.tile([128, 15, M], f32)
            nc.sync.dma_start(convl_sb[:], convl_d[:])
            convl_bf = wp.tile([128, 15, M], bf16)
            nc.vector.tensor_copy(convl_bf[:], convl_sb[:])
            w2bd_sb = wp.tile([M, M], f32)
            nc.sync.dma_start(w2bd_sb[:], w2bd_d[:])
            g_sb = wp.tile([M, OX, 128], f32)
            nc.sync.dma_start(g_sb[:], g_d[:])
            aux4_sb = wp.tile([4, 6, M], f32)
            nc.sync.dma_start(aux4_sb[:], aux4_d[:])
            b2m_sb = wp.tile([M, 1], f32)
            nc.sync.dma_start(b2m_sb[:], b2m_d[:])
            fc1b_sb = wp.tile([128, 1], f32)
            nc.sync.dma_start(fc1b_sb[:], fc1b_d[:])
            fc2wt_sb = wp.tile([128, 10], f32)
            nc.sync.dma_start(fc2wt_sb[:], fc2wt_d[:])
            fc2b_sb = wp.tile([1, 10], f32)
            nc.sync.dma_start(fc2b_sb[:], fc2b_d[:])
            gb3_sb = wp.tile([C, 2], f32)
            nc.sync.dma_start(gb3_sb[:], gb3_d[:])
            ones64_sb = wp.tile([1, BC], f32)
            nc.vector.memset(ones64_sb[:], 1.0)
            ones128_sb = wp.tile([128, 1], f32)
            nc.vector.memset(ones128_sb[:], 1.0)
            acc6 = wp.tile([128, 6], f32)
            nc.vector.memset(acc6[:], 0.0)

            # U accumulators: [m, ci, b, ox]
            u_sb = up.tile([M, C, BC, OX], f32)

            # ---- main pass over batch chunks ----
            with tc.tile_pool(name="pscv", bufs=3, space="PSUM") as pscv, \
                 tc.tile_pool(name="xbp", bufs=2) as xbp:
                for c in range(NCHUNK):
                    x_sb = xp.tile([128, BCH, C, 130], f32)
                    nc.vector.memset(x_sb[:, :, :, 0:1], 0.0)
                    nc.vector.memset(x_sb[:, :, :, 129:130], 0.0)
                    nc.sync.dma_start(
                        x_sb[:, :, :, 1:129],
                        xc_d[c * BCH : (c + 1) * BCH].rearrange("b c h w -> h b c w"),
                    )
                    xbf = xbp.tile([128, BCH, C, 130], bf16)
                    nc.vector.tensor_copy(xbf[:], x_sb[:])
                    x5 = xbf.rearrange("h b c (ox f) -> h b c ox f", f=5)
                    cs = stp.tile([128, 6], f32)
                    for ci in range(C):
                        ps = pscv.tile([M, 512], f32)
                        out_ap = ps[:, 0 : BCH * OX].rearrange(
                            "m (b ox) -> m b ox", b=BCH
                        )
                        for kx in range(5):
                            nc.tensor.matmul(
                                out_ap,
                                convl_bf[:, ci * 5 + kx, :],
                                x5[:, :, ci, :, kx],
                                start=(kx == 0),
                                stop=(kx == 4),
                            )
                        nc.vector.tensor_copy(
                            u_sb[:, ci, c * BCH : (c + 1) * BCH, :], out_ap
                        )
                        xin = x_sb[:, :, ci, 1:129]
                        sq_scr = scrp.tile([128, BCH, 128], f32)
                        nc.scalar.activation(
                            sq_scr[:], xin, AF.Square,
                            accum_out=cs[:, 3 + ci : 4 + ci],
                        )
                        sx_scr = scrp.tile([128, BCH, 128], f32)
                        nc.vector.tensor_scalar(
                            sx_scr[:], xin, 0.0, 0.0, ALU.add, ALU.add,
                            accum_out=cs[:, ci : ci + 1],
                        )
                    nc.vector.tensor_add(acc6[:], acc6[:], cs[:])

            # ---- global stats: partition-reduce + AllReduce ----
            with tc.tile_pool(name="psst", bufs=1, space="PSUM") as psst:
                st_ps = psst.tile([6, 1], f32)
                nc.tensor.matmul(st_ps[:], acc6[:], ones128_sb[:], start=True, stop=True)
                st_sb = smallp.tile([8, 1], f32)
                nc.vector.memset(st_sb[:], 0.0)
                nc.vector.tensor_copy(st_sb[0:6, :], st_ps[:])

            g6 = smallp.tile([8, 1], f32)
            if _DEBUG_NO_COLLECTIVE:
                cc_in = dp.tile([8, 1], f32)
                cc_out = dp.tile([8, 1], f32)
                nc.sync.dma_start(cc_in[:], st_sb[:])
                nc.sync.dma_start(cc_out[:], cc_in[:])
                nc.sync.dma_start(g6[:], cc_out[:])
            else:
                # AllGather (bypass) + local matmul-sum: avoids the AllReduce
                # path that raises NRT_EXEC_UNIT_UNRECOVERABLE here.
                cc_in = dp.tile([8, 1], f32)
                cc_out = dp.tile([N_CORES * 8, 1], f32)
                nc.gpsimd.dma_start(cc_in[:], st_sb[:])
                nc.gpsimd.collective_compute(
                    "AllGather",
                    ALU.bypass,
                    replica_groups=[list(range(N_CORES))],
                    ins=[cc_in.opt()],
                    outs=[cc_out.opt()],
                )
                cg8 = smallp.tile([N_CORES, 8], f32)
                nc.sync.dma_start(
                    cg8[:], cc_out[:].rearrange("(r s) one -> r (s one)", r=N_CORES)
                )
                with tc.tile_pool(name="psag", bufs=1, space="PSUM") as psag:
                    g6ps = psag.tile([8, 8], f32)
                    nc.tensor.matmul(
                        g6ps[:, 0:1], cg8[:],
                        ones128_sb[0:8, :], start=True, stop=True,
                    )
                    nc.vector.tensor_copy(g6[:], g6ps[:, 0:1])

            lvl = _DEBUG_CUT if _DEBUG_CUT else 99
            if lvl <= 8:
                dummy = smallp.tile([BC, 10], f32)
                nc.vector.memset(dummy[:], 0.0)
                nc.sync.dma_start(out_d[:], dummy[:])

            # ---- s/t per channel ----
            if lvl > 1:
                ex6 = smallp.tile([6, 1], f32)
                nc.scalar.activation(ex6[:], g6[0:6, :], AF.Copy, bias=0.0, scale=INV_N)
                # compute engines need partition-start % 32 == 0; DMA rows 3:6 down
                esq3 = smallp.tile([C, 1], f32)
                nc.sync.dma_start(esq3[:], ex6[3:6, :])
                msq = smallp.tile([C, 1], f32)
                nc.vector.tensor_mul(msq[:], ex6[0:3, :], ex6[0:3, :])
                var3 = smallp.tile([C, 1], f32)
                nc.vector.tensor_sub(var3[:], esq3[:], msq[:])
                eps_sb = smallp.tile([C, 1], f32)
                nc.vector.memset(eps_sb[:], EPS)
                std3 = smallp.tile([C, 1], f32)
                nc.scalar.activation(std3[:], var3[:], AF.Sqrt, bias=eps_sb[:])
                rinv3 = smallp.tile([C, 1], f32)
                nc.vector.reciprocal(rinv3[:], std3[:])
                s3 = smallp.tile([C, 1], f32)
                nc.vector.tensor_mul(s3[:], rinv3[:], gb3_sb[:, 0:1])
                mt3 = smallp.tile([C, 1], f32)
                nc.vector.tensor_mul(mt3[:], ex6[0:3, :], s3[:])
                t3 = smallp.tile([C, 1], f32)
                nc.vector.tensor_sub(t3[:], gb3_sb[:, 1:2], mt3[:])
                rt4 = smallp.tile([4, 1], f32)
                nc.vector.memset(rt4[:], 1.0)
                nc.vector.tensor_copy(rt4[0:3, :], t3[:])
                rs4 = smallp.tile([4, 1], f32)
                nc.vector.memset(rs4[:], 1.0)
                nc.vector.tensor_copy(rs4[0:3, :], s3[:])

            # ---- broadcast vectors via K=4 matmuls ----
            # cols: 0=Tm_main 1=Tm_ox0 2=Tm_ox25 3..5 = s_bcast per ci
            if lvl > 2:
                vec_sb = smallp.tile([M, 6], f32)
                with tc.tile_pool(name="pstn", bufs=1, space="PSUM") as pstn:
                    tn_ps = pstn.tile([M, 8], f32)
                    for j in range(6):
                        rhs = rt4 if j < 3 else rs4
                        nc.tensor.matmul(
                            tn_ps[:, j : j + 1], aux4_sb[:, j, :], rhs[:],
                            start=True, stop=True,
                        )
                    nc.vector.tensor_copy(vec_sb[:], tn_ps[:, 0:6])

            # ---- assemble z1 = relu(y1) ----
            if lvl > 3:
                z1 = up.tile([M, BC, OX], f32)
                nc.vector.tensor_scalar(
                    z1[:], u_sb[:, 0], vec_sb[:, 3:4], vec_sb[:, 0:1], ALU.mult, ALU.add
                )
                nc.vector.scalar_tensor_tensor(
                    z1[:], u_sb[:, 1], vec_sb[:, 4:5], z1[:], ALU.mult, ALU.add
                )
                nc.vector.scalar_tensor_tensor(
                    z1[:], u_sb[:, 2], vec_sb[:, 5:6], z1[:], ALU.mult, ALU.add
                )
                nc.vector.tensor_scalar(
                    z1[:, :, 0:1], z1[:, :, 0:1], vec_sb[:, 1:2], None, ALU.add
                )
                nc.vector.tensor_scalar(
                    z1[:, :, 25:26], z1[:, :, 25:26], vec_sb[:, 2:3], None, ALU.add
                )
                nc.vector.tensor_scalar_max(z1[:], z1[:], 0.0)

            # ---- z2 = relu(W2 . z1 + b2) (1x1 conv over co) ----
            if lvl > 4:
                z2 = up.tile([M, BC, OX], f32)
                with tc.tile_pool(name="psw2", bufs=2, space="PSUM") as psw2:
                    for j in range(4):
                        pw = psw2.tile([M, 512], f32)
                        w_ap = pw[:, 0 : BCH * OX].rearrange("m (b ox) -> m b ox", b=BCH)
                        nc.tensor.matmul(
                            w_ap, w2bd_sb[:], z1[:, j * BCH : (j + 1) * BCH, :],
                            start=True, stop=True,
                        )
                        nc.scalar.activation(
                            z2[:, j * BCH : (j + 1) * BCH, :], w_ap, AF.Relu,
                            bias=b2m_sb[:],
                        )

            # ---- fc1 (gathered sparse cols) + relu ----
            if lvl > 5:
                f_sb = smallp.tile([128, BC], f32)
                with tc.tile_pool(name="psf", bufs=1, space="PSUM") as psf:
                    f_ps = psf.tile([128, BC], f32)
                    for ox in range(OX):
                        nc.tensor.matmul(
                            f_ps[:], g_sb[:, ox, :], z2[:, :, ox],
                            start=(ox == 0), stop=(ox == OX - 1),
                        )
                    nc.scalar.activation(f_sb[:], f_ps[:], AF.Relu, bias=fc1b_sb[:])

            # ---- fc2 + log_softmax ----
            if lvl > 6:
                with tc.tile_pool(name="pso", bufs=1, space="PSUM") as pso:
                    o_ps = pso.tile([BC, 16], f32)
                    lg = o_ps[:, 0:10]
                    nc.tensor.matmul(lg, f_sb[:], fc2wt_sb[:], start=True, stop=False)
                    nc.tensor.matmul(lg, ones64_sb[:], fc2b_sb[:], start=False, stop=True)
                    if lvl > 7:
                        mx = smallp.tile([BC, 1], f32)
                        nc.vector.tensor_reduce(mx[:], lg, AX.X, ALU.max)
                        nmx = smallp.tile([BC, 1], f32)
                        nc.scalar.activation(nmx[:], mx[:], AF.Copy, bias=0.0, scale=-1.0)
                        exps = smallp.tile([BC, 10], f32)
                        se = smallp.tile([BC, 1], f32)
                        nc.scalar.activation(exps[:], lg, AF.Exp, bias=nmx[:], accum_out=se[:])
                        ls = smallp.tile([BC, 1], f32)
                        nc.scalar.activation(ls[:], se[:], AF.Ln)
                    if lvl > 8:
                        out_sb = smallp.tile([BC, 10], f32)
                        nc.vector.tensor_scalar(
                            out_sb[:], lg, mx[:], ls[:], ALU.subtract, ALU.subtract
                        )
                        nc.sync.dma_start(out_d[:], out_sb[:])

    if not nc.is_finalized():
        nc.finalize()
    # Walrus allows at most 1 sem wait per instruction (2 on event sems).
    # Run the bacc split passes the BIR path doesn't apply by default.
    _bass_rust.move_matmul_waits_to_ldweights(nc.m)
    _bass_rust.generate_event_semaphores(nc)
    return nc


def _host_aux(w1, b1, w2, b2, fc1_w, fc1_b, fc2_w, fc2_b, gamma, beta):
    f = np.float32
    convl = np.zeros((128, 15, M), f)
    for ci in range(C):
        for kx in range(5):
            idx = ci * 5 + kx
            for oy in range(OY):
                for ky in range(5):
                    h = 5 * oy + ky - 1
                    if 0 <= h < 128:
                        convl[h, idx, 3 * oy : 3 * oy + 3] = w1[:, ci, ky, kx]

    w2bd = np.zeros((M, M), f)
    blk = w2[:, :, 0, 0].T.astype(f)  # [co', co]
    for oy in range(OY):
        w2bd[3 * oy : 3 * oy + 3, 3 * oy : 3 * oy + 3] = blk

    oyv = np.arange(OY)
    py = (5 * oyv) // 2
    px = py
    cols = (
        np.arange(C)[:, None, None] * 65 * 65
        + py[None, :, None] * 65
        + px[None, None, :]
    )  # [co, oy, ox]
    gmat = (
        fc1_w[:, cols].transpose(2, 1, 3, 0).reshape(M, OX, 128).astype(f)
    )  # [m=(oy,co), ox, k]

    full = w1.sum(axis=(2, 3))       # [co, ci]
    row0 = w1[:, :, 0, :].sum(-1)
    row4 = w1[:, :, 4, :].sum(-1)
    col0 = w1[:, :, :, 0].sum(-1)
    col4 = w1[:, :, :, 4].sum(-1)

    aux4 = np.zeros((4, 6, M), f)
    for m in range(M):
        oy, co = divmod(m, 3)
        for ci in range(C):
            aux4[ci, 0, m] = (
                full[co, ci]
                - (oy == 0) * row0[co, ci]
                - (oy == OY - 1) * row4[co, ci]
            )
            aux4[ci, 1, m] = (
                -col0[co, ci]
                + (oy == 0) * w1[co, ci, 0, 0]
                + (oy == OY - 1) * w1[co, ci, 4, 0]
            )
            aux4[ci, 2, m] = (
                -col4[co, ci]
                + (oy == 0) * w1[co, ci, 0, 4]
                + (oy == OY - 1) * w1[co, ci, 4, 4]
            )
            aux4[ci, 3 + ci, m] = 1.0
        aux4[3, 0, m] = b1[co]

    b2m = np.repeat(b2[None, :], OY, axis=0).reshape(M, 1).astype(f)
    fc1b = fc1_b.reshape(128, 1).astype(f)
    fc2wt = np.ascontiguousarray(fc2_w.T).astype(f)
    fc2b_a = fc2_b.reshape(1, 10).astype(f)
    gb3 = np.stack([gamma, beta], axis=1).astype(f)
    return dict(
        convl=np.ascontiguousarray(convl),
        w2bd=np.ascontiguousarray(w2bd),
        gmat=np.ascontiguousarray(gmat),
        aux4=np.ascontiguousarray(aux4),
        b2m=b2m,
        fc1b=fc1b,
        fc2wt=fc2wt,
        fc2b=fc2b_a,
        gb3=np.ascontiguousarray(gb3),
    )


def kernel(**inputs):
    global _PROG, LAST_EXEC_NS
    from concourse.bass_utils import run_bass_kernel_spmd

    x = np.asarray(inputs["x"], dtype=np.float32)
    aux = _host_aux(
        np.asarray(inputs["w1"], np.float32),
        np.asarray(inputs["b1"], np.float32),
        np.asarray(inputs["w2"], np.float32),
        np.asarray(inputs["b2"], np.float32),
        np.asarray(inputs["fc1_w"], np.float32),
        np.asarray(inputs["fc1_b"], np.float32),
        np.asarray(inputs["fc2_w"], np.float32),
        np.asarray(inputs["fc2_b"], np.float32),
        np.asarray(inputs["bn_gamma"], np.float32),
        np.asarray(inputs["bn_beta"], np.float32),
    )

    if _PROG is None:
        _PROG = _build_program()
    nc = _PROG

    in_maps = []
    for cid in range(N_CORES):
        m = {"xc": np.ascontiguousarray(x[cid * BC : (cid + 1) * BC])}
        m.update(aux)
        in_maps.append(m)

    trace = os.environ.get("BASS_KERNEL_TRACE") == "1"
    res = run_bass_kernel_spmd(nc, in_maps, list(range(N_CORES)), trace=trace)
    LAST_EXEC_NS = res.exec_time_ns
    out = np.concatenate([res.results[cid]["out"] for cid in range(N_CORES)], axis=0)
    return out.astype(np.float32)
